# revision 5
# baseline (speedup 1.0000x reference)
"""Expert-parallel MoE FFN for Trainium2 — two expert slots per NeuronCore.

Strategy
--------
The reference sums top-2 expert FFN outputs per token (binary combine mask).
Total work is TOKENS*TOP_K = 8192 token-expert pairs; a perfectly balanced
core processes 1024.  One-expert-per-core padding forces every core to the
max expert count (~1086 -> cap 1092).  Instead each core runs TWO slots
[A: q tokens | B: r tokens] with independent expert weight sets; a tiny
host-side solver picks (q, r) and an expert->slot assignment so that hot
experts span two A-slots and cold experts two B-slots.  For the graded
distribution this gives q=544, r=500, cap=1044 — 4.4% less PE time.

Device side per slot: hT = relu(W1^T-chained matmuls + b1); yT = W2-chained
matmuls + b2, contraction dims on SBUF partitions for both layers, bf16
inputs, fp32 PSUM.  Token columns are chunked <=512 to fit one PSUM bank.

DMA: both experts' weights (32 MB) stream through rotating SBUF pools in
exact consumption order on the single SWDGE (gpsimd) ring: W1A by f-groups,
W2A by m-groups (m-major so mm2's m-outer loop consumes groups in arrival
order), then W1B, W2B.  The first x/W1 groups ride the sync-queue HWDGE,
which starts generating ~3us earlier than the SWDGE ring, pulling the first
matmul from ~13.5us to ~9us.  Outputs ride the sync queue throughout.
"""

import numpy as np
import ml_dtypes

import concourse.bacc as bacc
import concourse.mybir as mybir
import concourse.tile as tile
from concourse.bass_utils import run_bass_kernel_spmd
from concourse._compat import get_trn_type

D_MODEL = 1024
D_FF = 4096
N_EXP = 8
TOP_K = 2
KD = D_MODEL // 128  # 8 contraction chunks over d_model
KF = D_FF // 128  # 32 contraction chunks over d_ff

# W1 f-column group breakpoints: small first groups so mm1 starts early.
W1_BREAKS_A = [0, 128, 384, 896, 1408, 1920, 2432, 2944, 3456, 3968, 4096]
W1_BREAKS_B = [0, 512, 1024, 1536, 2048, 2560, 3072, 3584, 4096]

BF16 = mybir.dt.bfloat16
F32 = mybir.dt.float32

_programs: dict[tuple, object] = {}


def _chunks(width):
    """Split a slot width into matmul chunks <=512 (one fp32 PSUM bank),
    each a multiple of 4 and >=128 (keeps LDWEIGHTS hidden)."""
    n = -(-width // 512)
    base = width // n
    sizes = []
    rem = width
    for i in range(n - 1):
        w = -(-base // 4) * 4
        sizes.append(w)
        rem -= w
    sizes.append(rem)
    assert sum(sizes) == width and all(
        128 <= w <= 512 and w % 4 == 0 for w in sizes
    ), (width, sizes)
    return sizes


def _build_program(q, r):
    """Bass/Tile program: two expert slots (q and r token columns)."""
    cap = q + r
    nc = bacc.Bacc(get_trn_type() or "TRN2", target_bir_lowering=False, debug=False)

    slots = [
        dict(key="A", width=q, breaks=W1_BREAKS_A),
        dict(key="B", width=r, breaks=W1_BREAKS_B),
    ]
    for s in slots:
        s["chunks"] = _chunks(s["width"])

    # ---- DRAM tensors ----------------------------------------------------
    ka = KD // 2
    xa0_d = nc.dram_tensor("xa0", [128, ka * slots[0]["chunks"][0]], BF16,
                           kind="ExternalInput").ap()
    xa1_d = nc.dram_tensor("xa1", [128, (KD - ka) * slots[0]["chunks"][0]], BF16,
                           kind="ExternalInput").ap()
    xa2_w = slots[0]["width"] - slots[0]["chunks"][0]
    xa2_d = nc.dram_tensor("xa2", [128, KD * xa2_w], BF16,
                           kind="ExternalInput").ap() if xa2_w else None
    xb_d = nc.dram_tensor("xb", [128, KD * slots[1]["width"]], BF16,
                          kind="ExternalInput").ap()
    for s in slots:
        k = s["key"]
        s["w1_d"] = [
            nc.dram_tensor(f"W1{k}{g}", [128, KD * (hi - lo)], BF16,
                           kind="ExternalInput").ap()
            for g, (lo, hi) in enumerate(zip(s["breaks"], s["breaks"][1:]))
        ]
        s["w2_d"] = [
            nc.dram_tensor(f"W2{k}{m}", [128, KF * 128], BF16,
                           kind="ExternalInput").ap()
            for m in range(KD)
        ]
        s["b1_d"] = nc.dram_tensor(f"b1{k}", [128, KF], F32, kind="ExternalInput").ap()
        s["b2_d"] = nc.dram_tensor(f"b2{k}", [128, KD], F32, kind="ExternalInput").ap()
    y_d = nc.dram_tensor("yT", [128, KD * cap], F32, kind="ExternalOutput").ap()
    y_v = y_d.rearrange("p (m c) -> p m c", c=cap)

    with tile.TileContext(nc) as tc:
        with (
            tc.tile_pool(name="sb", bufs=1) as sb,
            tc.tile_pool(name="wp", bufs=1) as wp,
            tc.tile_pool(name="hp", bufs=1) as hp,
            tc.tile_pool(name="yp", bufs=3) as yp,
            tc.tile_pool(name="pp1", bufs=5, space="PSUM") as pp1,
            tc.tile_pool(name="pp2", bufs=2, space="PSUM") as pp2,
        ):
            # ---- persistent input tiles ---------------------------------
            xa0 = sb.tile([128, xa0_d.shape[1]], BF16, tag="xa0", name="xa0_sb")
            xa1 = sb.tile([128, xa1_d.shape[1]], BF16, tag="xa1", name="xa1_sb")
            xa2 = (sb.tile([128, xa2_d.shape[1]], BF16, tag="xa2", name="xa2_sb")
                   if xa2_d is not None else None)
            xb = sb.tile([128, xb_d.shape[1]], BF16, tag="xb", name="xb_sb")
            for s in slots:
                k = s["key"]
                s["b1"] = sb.tile([128, KF], F32, tag=f"b1{k}", name=f"b1{k}_sb")
                s["b2"] = sb.tile([128, KD], F32, tag=f"b2{k}", name=f"b2{k}_sb")

            # Streaming weight tiles: rotating slots in the wp pool.
            w1_slot_cols = KD * max(
                hi - lo for s in slots
                for lo, hi in zip(s["breaks"], s["breaks"][1:])
            )
            for s in slots:
                s["w1_t"] = [
                    wp.tile([128, KD * (hi - lo)], BF16, tag="w1", bufs=10,
                            padded_shape=[128, w1_slot_cols],
                            name=f"w1{s['key']}{g}")
                    for g, (lo, hi) in enumerate(zip(s["breaks"], s["breaks"][1:]))
                ]
                s["w2_t"] = [
                    wp.tile([128, KF * 128], BF16, tag="w2", bufs=4,
                            name=f"w2{s['key']}{m}")
                    for m in range(KD)
                ]

            # ---- DMA: first operands on the early-starting sync queue ---
            nc.sync.dma_start(xa0[:], xa0_d)
            nc.sync.dma_start(slots[0]["w1_t"][0][:], slots[0]["w1_d"][0])
            nc.sync.dma_start(xa1[:], xa1_d)
            # Everything else rides the single SWDGE ring in need-order.
            nc.gpsimd.dma_start(slots[0]["w1_t"][1][:], slots[0]["w1_d"][1])
            if xa2 is not None:
                nc.gpsimd.dma_start(xa2[:], xa2_d)
            nc.gpsimd.dma_start(slots[0]["b1"][:], slots[0]["b1_d"])
            nc.gpsimd.dma_start(slots[0]["b2"][:], slots[0]["b2_d"])
            for g in range(2, len(slots[0]["w1_t"])):
                nc.gpsimd.dma_start(slots[0]["w1_t"][g][:], slots[0]["w1_d"][g])
            for m in range(KD):
                nc.gpsimd.dma_start(slots[0]["w2_t"][m][:], slots[0]["w2_d"][m])
            nc.gpsimd.dma_start(slots[1]["b1"][:], slots[1]["b1_d"])
            nc.gpsimd.dma_start(slots[1]["b2"][:], slots[1]["b2_d"])
            nc.gpsimd.dma_start(xb[:], xb_d)
            for g in range(len(slots[1]["w1_t"])):
                nc.gpsimd.dma_start(slots[1]["w1_t"][g][:], slots[1]["w1_d"][g])
            for m in range(KD):
                nc.gpsimd.dma_start(slots[1]["w2_t"][m][:], slots[1]["w2_d"][m])

            # ---- accessors ----------------------------------------------
            def x_rhs(s, k, off, w):
                if s["key"] == "B":
                    return xb[:, k * s["width"] + off : k * s["width"] + off + w]
                c0 = s["chunks"][0]
                if off < c0:
                    t, kk = (xa0, k) if k < ka else (xa1, k - ka)
                    return t[:, kk * c0 + off : kk * c0 + off + w]
                return xa2[:, k * xa2_w + (off - c0) : k * xa2_w + (off - c0) + w]

            def w1_lhsT(s, k, f):
                col = f * 128
                for (lo, hi), t in zip(zip(s["breaks"], s["breaks"][1:]), s["w1_t"]):
                    if lo <= col < hi:
                        base = k * (hi - lo) + (col - lo)
                        return t[:, base : base + 128]
                raise AssertionError

            # ---- compute: per slot, mm1 then mm2 ------------------------
            cap_off = 0
            for s in slots:
                offs = []
                o = 0
                for w in s["chunks"]:
                    offs.append(o)
                    o += w
                # mm1 runs chunk-by-chunk (not interleaved) so the very first
                # matmuls depend only on the sync-queue groups (xa0/W1g0/xa1);
                # chunk 1's x block has a full chunk-0 f-sweep of slack.
                h_tiles = [[None] * KF for _ in s["chunks"]]
                for ci, (o, w) in enumerate(zip(offs, s["chunks"])):
                    for f in range(KF):
                        ps = pp1.tile([128, 512], F32, tag="ps1", bufs=5,
                                      name=f"ps1_{s['key']}_{ci}_{f}")
                        for k in range(KD):
                            nc.tensor.matmul(
                                ps[:, :w],
                                w1_lhsT(s, k, f),
                                x_rhs(s, k, o, w),
                                start=(k == 0),
                                stop=(k == KD - 1),
                            )
                        ht = hp.tile([128, w], BF16, tag=f"h{s['key']}{ci}",
                                     bufs=KF, name=f"h_{s['key']}_{ci}_{f}")
                        nc.scalar.activation(
                            ht[:],
                            ps[:, :w],
                            mybir.ActivationFunctionType.Relu,
                            bias=s["b1"][:, f : f + 1],
                        )
                        h_tiles[ci][f] = ht

                for m in range(KD):
                    ps2s = [
                        pp2.tile([128, 512], F32, tag="ps2", bufs=2,
                                 name=f"ps2_{s['key']}_{m}_{ci}")
                        for ci in range(len(s["chunks"]))
                    ]
                    for f in range(KF):
                        w2t = s["w2_t"][m]
                        for ci, w in enumerate(s["chunks"]):
                            nc.tensor.matmul(
                                ps2s[ci][:, :w],
                                w2t[:, f * 128 : (f + 1) * 128],
                                h_tiles[ci][f][:],
                                start=(f == 0),
                                stop=(f == KF - 1),
                            )
                    yt = yp.tile([128, s["width"]], F32, tag="y",
                                 padded_shape=[128, max(q, r)],
                                 name=f"y_{s['key']}_{m}")
                    for ci, (o, w) in enumerate(zip(offs, s["chunks"])):
                        nc.vector.tensor_scalar_add(
                            yt[:, o : o + w], ps2s[ci][:, :w], s["b2"][:, m : m + 1]
                        )
                    nc.sync.dma_start(
                        y_v[:, m, cap_off : cap_off + s["width"]], yt[:, : s["width"]]
                    )
                cap_off += s["width"]

    nc.compile()
    return nc


def _gating_topk(x, Wg, bg):
    """Replicates jax.nn.softmax + jax.lax.top_k(..., 2) in fp32 numpy."""
    logits = x @ Wg + bg
    m = logits.max(axis=1, keepdims=True)
    e = np.exp(logits - m)
    scores = e / e.sum(axis=1, keepdims=True)
    order = np.argsort(-scores, axis=1, kind="stable")
    return order[:, :TOP_K]


def _solve_slots(counts):
    """Pick slot widths (q, r) and an expert assignment minimizing cap=q+r.

    Each core runs [q cols of expert a_c | r cols of expert b_c].  Expert e
    may occupy several A- and/or B-slots (n_A, n_B) as long as
    n_A*q + n_B*r >= counts[e], sum(n_A) <= 8, sum(n_B) <= 8.
    Returns (q, r, a_slots, b_slots): lists of 8 expert ids.
    """
    E = len(counts)
    cmax = max(counts)

    def feasible(qq, rr):
        # DP over experts on (sumA, sumB); keep pareto frontier with choices.
        frontier = {(0, 0): []}
        for e in range(E):
            new = {}
            for na in range(0, E + 1):
                rem = counts[e] - na * qq
                nb = 0 if rem <= 0 else (-(-rem // rr) if rr > 0 else None)
                if nb is None or nb > E:
                    continue
                for (sa, sb), ch in frontier.items():
                    key = (sa + na, sb + nb)
                    if key[0] <= E and key[1] <= E and key not in new:
                        new[key] = ch + [(na, nb)]
            # prune dominated states
            pruned = {}
            for k in sorted(new):
                if not any(k2[0] <= k[0] and k2[1] <= k[1] and k2 != k
                           for k2 in new):
                    pruned[k] = new[k]
            frontier = pruned or new
            if not frontier:
                return None
        best = min(((sa, sb) for sa, sb in frontier if sa <= 8 and sb <= 8),
                   default=None)
        return frontier[best] if best else None

    up4 = lambda v: -(-v // 4) * 4
    fallback = up4(-(-cmax // 2))
    for cap in range(up4(-(-sum(counts) // E)), 2 * fallback + 8, 4):
        for qq in range(up4(-(-cap // 2)), cap - 127, 4):
            rr = cap - qq
            if rr < 128 or qq > 1024 or rr > 1024:
                continue
            choice = feasible(qq, rr)
            if choice:
                a_slots, b_slots = [], []
                for e, (na, nb) in enumerate(choice):
                    a_slots += [e] * na
                    b_slots += [e] * nb
                a_slots += [a_slots[-1]] * (8 - len(a_slots))
                b_slots += [b_slots[-1]] * (8 - len(b_slots))
                return qq, rr, a_slots, b_slots
    # guaranteed fallback: every core single-expert across both slots
    q = r = fallback
    return q, r, list(range(E)), list(range(E))


def _pack_k128(a):
    """[K*128, F] -> [128, K*F]: partition-major packing of the SBUF layout."""
    k128, f = a.shape
    return np.ascontiguousarray(
        a.reshape(k128 // 128, 128, f).transpose(1, 0, 2).reshape(128, -1)
    )


def _prepare(x, Wg, bg, W1, b1, W2, b2):
    x = np.ascontiguousarray(np.asarray(x, dtype=np.float32))
    topk = _gating_topk(x, np.asarray(Wg, np.float32), np.asarray(bg, np.float32))
    idx = [np.nonzero((topk == e).any(axis=1))[0] for e in range(N_EXP)]
    counts = [len(i) for i in idx]
    q, r, a_slots, b_slots = _solve_slots(counts)

    # Distribute each expert's tokens over its slots (A-slots first).
    slot_tokens = {}  # (core, 'A'|'B') -> token ids
    cursor = [0] * N_EXP
    for key, width, owners in (("A", q, a_slots), ("B", r, b_slots)):
        for c, e in enumerate(owners):
            t0 = cursor[e]
            t1 = min(t0 + width, counts[e])
            slot_tokens[(c, key)] = idx[e][t0:t1]
            cursor[e] = t1
    assert all(cursor[e] >= counts[e] for e in range(N_EXP)), (
        counts, q, r, a_slots, b_slots)

    bf16 = ml_dtypes.bfloat16
    w1packs, w2packs = {}, {}
    for e in set(a_slots) | set(b_slots):
        w1 = np.asarray(W1[e], np.float32).astype(bf16)
        w1packs[e] = _pack_k128(w1).reshape(128, KD, D_FF)
        w2 = np.asarray(W2[e], np.float32).astype(bf16)
        w2packs[e] = _pack_k128(w2).reshape(128, KF, D_MODEL)

    chunks_a = _chunks(q)
    ka = KD // 2
    xa2_w = q - chunks_a[0]
    in_maps = []
    for c in range(N_EXP):
        m = {}
        for key, width, owners, breaks in (
            ("A", q, a_slots, W1_BREAKS_A),
            ("B", r, b_slots, W1_BREAKS_B),
        ):
            e = owners[c]
            toks = slot_tokens[(c, key)]
            xg = np.zeros((width, D_MODEL), np.float32)
            xg[: len(toks)] = x[toks]
            xT = np.ascontiguousarray(xg.T).astype(bf16)
            xTp = _pack_k128(xT).reshape(128, KD, width)
            if key == "A":
                c0 = chunks_a[0]
                m["xa0"] = np.ascontiguousarray(xTp[:, :ka, :c0]).reshape(128, -1)
                m["xa1"] = np.ascontiguousarray(xTp[:, ka:, :c0]).reshape(128, -1)
                if xa2_w:
                    m["xa2"] = np.ascontiguousarray(xTp[:, :, c0:]).reshape(128, -1)
            else:
                m["xb"] = np.ascontiguousarray(xTp).reshape(128, -1)
            for g, (lo, hi) in enumerate(zip(breaks, breaks[1:])):
                m[f"W1{key}{g}"] = np.ascontiguousarray(
                    w1packs[e][:, :, lo:hi]
                ).reshape(128, -1)
            for mm in range(KD):
                m[f"W2{key}{mm}"] = np.ascontiguousarray(
                    w2packs[e][:, :, mm * 128 : (mm + 1) * 128]
                ).reshape(128, -1)
            m[f"b1{key}"] = np.ascontiguousarray(
                np.asarray(b1[e], np.float32).reshape(KF, 128).T
            )
            m[f"b2{key}"] = np.ascontiguousarray(
                np.asarray(b2[e], np.float32).reshape(KD, 128).T
            )
        in_maps.append(m)
    return x, slot_tokens, q, r, in_maps


def _run(x, Wg, bg, W1, b1, W2, b2, **run_kwargs):
    x, slot_tokens, q, r, in_maps = _prepare(x, Wg, bg, W1, b1, W2, b2)
    cap = q + r
    key = (q, r)
    prog = _programs.get(key)
    if prog is None:
        prog = _programs.setdefault(key, _build_program(q, r))
    res = run_bass_kernel_spmd(
        prog, in_maps, core_ids=list(range(N_EXP)), **run_kwargs
    )
    out = np.zeros_like(x)
    for c in range(N_EXP):
        yp = np.asarray(res.results[c]["yT"], np.float32)  # [128, KD*cap]
        yT = yp.reshape(128, KD, cap).transpose(1, 0, 2).reshape(D_MODEL, cap)
        for key2, off, width in (("A", 0, q), ("B", q, r)):
            toks = slot_tokens[(c, key2)]
            out[toks] += yT[:, off : off + len(toks)].T
    return out, res


def kernel(x, Wg, bg, W1, b1, W2, b2):
    out, _ = _run(x, Wg, bg, W1, b1, W2, b2)
    return out


# revision 6
# speedup vs baseline: 1.1651x; 1.1651x over previous
"""Expert-parallel MoE FFN for Trainium2 — two expert slots per NeuronCore.

Strategy
--------
The reference sums top-2 expert FFN outputs per token (binary combine mask).
Total work is TOKENS*TOP_K = 8192 token-expert pairs; a perfectly balanced
core processes 1024.  One-expert-per-core padding forces every core to the
max expert count (~1086 -> cap 1092).  Instead each core runs TWO slots
[A: q tokens | B: r tokens] with independent expert weight sets; a tiny
host-side solver picks (q, r) and an expert->slot assignment so that hot
experts span two A-slots and cold experts two B-slots.  For the graded
distribution this gives q=544, r=500, cap=1044 — 4.4% less PE time.

Device side per slot: hT = relu(W1^T-chained matmuls + b1); yT = W2-chained
matmuls + b2, contraction dims on SBUF partitions for both layers, bf16
inputs, fp32 PSUM.  Token columns are chunked <=512 to fit one PSUM bank.

DMA: both experts' weights (32 MB) stream through rotating SBUF pools in
exact consumption order on the single SWDGE (gpsimd) ring: W1A by f-groups,
W2A by m-groups (m-major so mm2's m-outer loop consumes groups in arrival
order), then W1B, W2B.  The first x/W1 groups ride the sync-queue HWDGE,
which starts generating ~3us earlier than the SWDGE ring, pulling the first
matmul from ~13.5us to ~9us.  Outputs ride the sync queue throughout.
"""

import numpy as np
import ml_dtypes

import concourse.bacc as bacc
import concourse.mybir as mybir
import concourse.tile as tile
from concourse.bass_utils import run_bass_kernel_spmd
from concourse._compat import get_trn_type

D_MODEL = 1024
D_FF = 4096
N_EXP = 8
TOP_K = 2
KD = D_MODEL // 128  # 8 contraction chunks over d_model
KF = D_FF // 128  # 32 contraction chunks over d_ff

# W1 f-column group breakpoints: small first groups so mm1 starts early.
W1_BREAKS_A = [0, 128, 384, 896, 1408, 1920, 2432, 2944, 3456, 3968, 4096]
W1_BREAKS_B = [0, 512, 1024, 1536, 2048, 2560, 3072, 3584, 4096]

BF16 = mybir.dt.bfloat16
F32 = mybir.dt.float32

_programs: dict[tuple, object] = {}


def _chunks(width):
    """Split a slot width into matmul chunks <=512 (one fp32 PSUM bank),
    each a multiple of 4 and >=128 (keeps LDWEIGHTS hidden)."""
    n = -(-width // 512)
    base = width // n
    sizes = []
    rem = width
    for i in range(n - 1):
        w = -(-base // 4) * 4
        sizes.append(w)
        rem -= w
    sizes.append(rem)
    assert sum(sizes) == width and all(
        128 <= w <= 512 and w % 4 == 0 for w in sizes
    ), (width, sizes)
    return sizes


def _build_program(q, r):
    """Bass/Tile program: two expert slots (q and r token columns)."""
    cap = q + r
    nc = bacc.Bacc(get_trn_type() or "TRN2", target_bir_lowering=False, debug=False)

    slots = [
        dict(key="A", width=q, breaks=W1_BREAKS_A),
        dict(key="B", width=r, breaks=W1_BREAKS_B),
    ]
    for s in slots:
        s["chunks"] = _chunks(s["width"])

    # ---- DRAM tensors ----------------------------------------------------
    ka = KD // 2
    xa0_d = nc.dram_tensor("xa0", [128, ka * slots[0]["chunks"][0]], BF16,
                           kind="ExternalInput").ap()
    xa1_d = nc.dram_tensor("xa1", [128, (KD - ka) * slots[0]["chunks"][0]], BF16,
                           kind="ExternalInput").ap()
    xa2_w = slots[0]["width"] - slots[0]["chunks"][0]
    xa2_d = nc.dram_tensor("xa2", [128, KD * xa2_w], BF16,
                           kind="ExternalInput").ap() if xa2_w else None
    xb_d = nc.dram_tensor("xb", [128, KD * slots[1]["width"]], BF16,
                          kind="ExternalInput").ap()
    for s in slots:
        k = s["key"]
        s["w1_d"] = [
            nc.dram_tensor(f"W1{k}{g}", [128, KD * (hi - lo)], BF16,
                           kind="ExternalInput").ap()
            for g, (lo, hi) in enumerate(zip(s["breaks"], s["breaks"][1:]))
        ]
        s["w2_d"] = [
            nc.dram_tensor(f"W2{k}{m}", [128, KF * 128], BF16,
                           kind="ExternalInput").ap()
            for m in range(KD)
        ]
        s["b1_d"] = nc.dram_tensor(f"b1{k}", [128, KF], F32, kind="ExternalInput").ap()
        s["b2_d"] = nc.dram_tensor(f"b2{k}", [128, KD], F32, kind="ExternalInput").ap()
    y_d = nc.dram_tensor("yT", [128, KD * cap], F32, kind="ExternalOutput").ap()
    y_v = y_d.rearrange("p (m c) -> p m c", c=cap)

    with tile.TileContext(nc) as tc:
        with (
            tc.tile_pool(name="sb", bufs=1) as sb,
            tc.tile_pool(name="wp", bufs=1) as wp,
            tc.tile_pool(name="hp", bufs=1) as hp,
            tc.tile_pool(name="yp", bufs=3) as yp,
            tc.tile_pool(name="pp1", bufs=5, space="PSUM") as pp1,
            tc.tile_pool(name="pp2", bufs=2, space="PSUM") as pp2,
        ):
            # ---- persistent input tiles ---------------------------------
            xa0 = sb.tile([128, xa0_d.shape[1]], BF16, tag="xa0", name="xa0_sb")
            xa1 = sb.tile([128, xa1_d.shape[1]], BF16, tag="xa1", name="xa1_sb")
            xa2 = (sb.tile([128, xa2_d.shape[1]], BF16, tag="xa2", name="xa2_sb")
                   if xa2_d is not None else None)
            xb = sb.tile([128, xb_d.shape[1]], BF16, tag="xb", name="xb_sb")
            for s in slots:
                k = s["key"]
                s["b1"] = sb.tile([128, KF], F32, tag=f"b1{k}", name=f"b1{k}_sb")
                s["b2"] = sb.tile([128, KD], F32, tag=f"b2{k}", name=f"b2{k}_sb")

            # Streaming weight tiles: rotating slots in the wp pool.
            w1_slot_cols = KD * max(
                hi - lo for s in slots
                for lo, hi in zip(s["breaks"], s["breaks"][1:])
            )
            for s in slots:
                s["w1_t"] = [
                    wp.tile([128, KD * (hi - lo)], BF16, tag="w1", bufs=10,
                            padded_shape=[128, w1_slot_cols],
                            name=f"w1{s['key']}{g}")
                    for g, (lo, hi) in enumerate(zip(s["breaks"], s["breaks"][1:]))
                ]
                s["w2_t"] = [
                    wp.tile([128, KF * 128], BF16, tag="w2", bufs=4,
                            name=f"w2{s['key']}{m}")
                    for m in range(KD)
                ]

            # ---- DMA: first operands on the early-starting sync queue ---
            nc.sync.dma_start(xa0[:], xa0_d)
            nc.sync.dma_start(slots[0]["w1_t"][0][:], slots[0]["w1_d"][0])
            nc.sync.dma_start(xa1[:], xa1_d)
            # Everything else rides the single SWDGE ring in need-order.
            nc.gpsimd.dma_start(slots[0]["w1_t"][1][:], slots[0]["w1_d"][1])
            if xa2 is not None:
                nc.gpsimd.dma_start(xa2[:], xa2_d)
            nc.gpsimd.dma_start(slots[0]["b1"][:], slots[0]["b1_d"])
            nc.gpsimd.dma_start(slots[0]["b2"][:], slots[0]["b2_d"])
            for g in range(2, len(slots[0]["w1_t"])):
                nc.gpsimd.dma_start(slots[0]["w1_t"][g][:], slots[0]["w1_d"][g])
            for m in range(KD):
                nc.gpsimd.dma_start(slots[0]["w2_t"][m][:], slots[0]["w2_d"][m])
            nc.gpsimd.dma_start(slots[1]["b1"][:], slots[1]["b1_d"])
            nc.gpsimd.dma_start(slots[1]["b2"][:], slots[1]["b2_d"])
            nc.gpsimd.dma_start(xb[:], xb_d)
            for g in range(len(slots[1]["w1_t"])):
                nc.gpsimd.dma_start(slots[1]["w1_t"][g][:], slots[1]["w1_d"][g])
            for m in range(KD):
                nc.gpsimd.dma_start(slots[1]["w2_t"][m][:], slots[1]["w2_d"][m])

            # ---- accessors ----------------------------------------------
            def x_rhs(s, k, off, w):
                if s["key"] == "B":
                    return xb[:, k * s["width"] + off : k * s["width"] + off + w]
                c0 = s["chunks"][0]
                if off < c0:
                    t, kk = (xa0, k) if k < ka else (xa1, k - ka)
                    return t[:, kk * c0 + off : kk * c0 + off + w]
                return xa2[:, k * xa2_w + (off - c0) : k * xa2_w + (off - c0) + w]

            def w1_lhsT(s, k, f):
                col = f * 128
                for (lo, hi), t in zip(zip(s["breaks"], s["breaks"][1:]), s["w1_t"]):
                    if lo <= col < hi:
                        base = k * (hi - lo) + (col - lo)
                        return t[:, base : base + 128]
                raise AssertionError

            # ---- compute: per slot, mm1 then mm2 ------------------------
            cap_off = 0
            for s in slots:
                offs = []
                o = 0
                for w in s["chunks"]:
                    offs.append(o)
                    o += w
                # mm1 runs chunk-by-chunk (not interleaved) so the very first
                # matmuls depend only on the sync-queue groups (xa0/W1g0/xa1);
                # chunk 1's x block has a full chunk-0 f-sweep of slack.
                h_tiles = [[None] * KF for _ in s["chunks"]]
                for ci, (o, w) in enumerate(zip(offs, s["chunks"])):
                    for f in range(KF):
                        ps = pp1.tile([128, w], F32, tag="ps1", bufs=5,
                                      name=f"ps1_{s['key']}_{ci}_{f}")
                        for k in range(KD):
                            nc.tensor.matmul(
                                ps[:],
                                w1_lhsT(s, k, f),
                                x_rhs(s, k, o, w),
                                start=(k == 0),
                                stop=(k == KD - 1),
                            )
                        ht = hp.tile([128, w], BF16, tag=f"h{s['key']}{ci}",
                                     bufs=KF, name=f"h_{s['key']}_{ci}_{f}")
                        nc.scalar.activation(
                            ht[:],
                            ps[:],
                            mybir.ActivationFunctionType.Relu,
                            bias=s["b1"][:, f : f + 1],
                        )
                        h_tiles[ci][f] = ht

                for m in range(KD):
                    ps2s = [
                        pp2.tile([128, w], F32, tag="ps2", bufs=2,
                                 name=f"ps2_{s['key']}_{m}_{ci}")
                        for ci, w in enumerate(s["chunks"])
                    ]
                    for f in range(KF):
                        w2t = s["w2_t"][m]
                        for ci, w in enumerate(s["chunks"]):
                            nc.tensor.matmul(
                                ps2s[ci][:],
                                w2t[:, f * 128 : (f + 1) * 128],
                                h_tiles[ci][f][:],
                                start=(f == 0),
                                stop=(f == KF - 1),
                            )
                    yt = yp.tile([128, s["width"]], F32, tag="y",
                                 padded_shape=[128, max(q, r)],
                                 name=f"y_{s['key']}_{m}")
                    for ci, (o, w) in enumerate(zip(offs, s["chunks"])):
                        nc.vector.tensor_scalar_add(
                            yt[:, o : o + w], ps2s[ci][:], s["b2"][:, m : m + 1]
                        )
                    nc.sync.dma_start(
                        y_v[:, m, cap_off : cap_off + s["width"]], yt[:, : s["width"]]
                    )
                cap_off += s["width"]

    nc.compile()
    return nc


def _gating_topk(x, Wg, bg):
    """Replicates jax.nn.softmax + jax.lax.top_k(..., 2) in fp32 numpy."""
    logits = x @ Wg + bg
    m = logits.max(axis=1, keepdims=True)
    e = np.exp(logits - m)
    scores = e / e.sum(axis=1, keepdims=True)
    order = np.argsort(-scores, axis=1, kind="stable")
    return order[:, :TOP_K]


def _solve_slots(counts):
    """Pick slot widths (q, r) and an expert assignment minimizing cap=q+r.

    Each core runs [q cols of expert a_c | r cols of expert b_c].  Expert e
    may occupy several A- and/or B-slots (n_A, n_B) as long as
    n_A*q + n_B*r >= counts[e], sum(n_A) <= 8, sum(n_B) <= 8.
    Returns (q, r, a_slots, b_slots): lists of 8 expert ids.
    """
    E = len(counts)
    cmax = max(counts)

    def feasible(qq, rr):
        # DP over experts on (sumA, sumB); keep pareto frontier with choices.
        frontier = {(0, 0): []}
        for e in range(E):
            new = {}
            for na in range(0, E + 1):
                rem = counts[e] - na * qq
                nb = 0 if rem <= 0 else (-(-rem // rr) if rr > 0 else None)
                if nb is None or nb > E:
                    continue
                for (sa, sb), ch in frontier.items():
                    key = (sa + na, sb + nb)
                    if key[0] <= E and key[1] <= E and key not in new:
                        new[key] = ch + [(na, nb)]
            # prune dominated states
            pruned = {}
            for k in sorted(new):
                if not any(k2[0] <= k[0] and k2[1] <= k[1] and k2 != k
                           for k2 in new):
                    pruned[k] = new[k]
            frontier = pruned or new
            if not frontier:
                return None
        best = min(((sa, sb) for sa, sb in frontier if sa <= 8 and sb <= 8),
                   default=None)
        return frontier[best] if best else None

    up4 = lambda v: -(-v // 4) * 4
    fallback = up4(-(-cmax // 2))
    for cap in range(up4(-(-sum(counts) // E)), 2 * fallback + 8, 4):
        for qq in range(up4(-(-cap // 2)), cap - 127, 4):
            rr = cap - qq
            if rr < 128 or qq > 1024 or rr > 1024:
                continue
            choice = feasible(qq, rr)
            if choice:
                a_slots, b_slots = [], []
                for e, (na, nb) in enumerate(choice):
                    a_slots += [e] * na
                    b_slots += [e] * nb
                a_slots += [a_slots[-1]] * (8 - len(a_slots))
                b_slots += [b_slots[-1]] * (8 - len(b_slots))
                return qq, rr, a_slots, b_slots
    # guaranteed fallback: every core single-expert across both slots
    q = r = fallback
    return q, r, list(range(E)), list(range(E))


def _pack_k128(a):
    """[K*128, F] -> [128, K*F]: partition-major packing of the SBUF layout."""
    k128, f = a.shape
    return np.ascontiguousarray(
        a.reshape(k128 // 128, 128, f).transpose(1, 0, 2).reshape(128, -1)
    )


def _prepare(x, Wg, bg, W1, b1, W2, b2):
    x = np.ascontiguousarray(np.asarray(x, dtype=np.float32))
    topk = _gating_topk(x, np.asarray(Wg, np.float32), np.asarray(bg, np.float32))
    idx = [np.nonzero((topk == e).any(axis=1))[0] for e in range(N_EXP)]
    counts = [len(i) for i in idx]
    q, r, a_slots, b_slots = _solve_slots(counts)

    # Distribute each expert's tokens over its slots (A-slots first).
    slot_tokens = {}  # (core, 'A'|'B') -> token ids
    cursor = [0] * N_EXP
    for key, width, owners in (("A", q, a_slots), ("B", r, b_slots)):
        for c, e in enumerate(owners):
            t0 = cursor[e]
            t1 = min(t0 + width, counts[e])
            slot_tokens[(c, key)] = idx[e][t0:t1]
            cursor[e] = t1
    assert all(cursor[e] >= counts[e] for e in range(N_EXP)), (
        counts, q, r, a_slots, b_slots)

    bf16 = ml_dtypes.bfloat16
    w1packs, w2packs = {}, {}
    for e in set(a_slots) | set(b_slots):
        w1 = np.asarray(W1[e], np.float32).astype(bf16)
        w1packs[e] = _pack_k128(w1).reshape(128, KD, D_FF)
        w2 = np.asarray(W2[e], np.float32).astype(bf16)
        w2packs[e] = _pack_k128(w2).reshape(128, KF, D_MODEL)

    chunks_a = _chunks(q)
    ka = KD // 2
    xa2_w = q - chunks_a[0]
    in_maps = []
    for c in range(N_EXP):
        m = {}
        for key, width, owners, breaks in (
            ("A", q, a_slots, W1_BREAKS_A),
            ("B", r, b_slots, W1_BREAKS_B),
        ):
            e = owners[c]
            toks = slot_tokens[(c, key)]
            xg = np.zeros((width, D_MODEL), np.float32)
            xg[: len(toks)] = x[toks]
            xT = np.ascontiguousarray(xg.T).astype(bf16)
            xTp = _pack_k128(xT).reshape(128, KD, width)
            if key == "A":
                c0 = chunks_a[0]
                m["xa0"] = np.ascontiguousarray(xTp[:, :ka, :c0]).reshape(128, -1)
                m["xa1"] = np.ascontiguousarray(xTp[:, ka:, :c0]).reshape(128, -1)
                if xa2_w:
                    m["xa2"] = np.ascontiguousarray(xTp[:, :, c0:]).reshape(128, -1)
            else:
                m["xb"] = np.ascontiguousarray(xTp).reshape(128, -1)
            for g, (lo, hi) in enumerate(zip(breaks, breaks[1:])):
                m[f"W1{key}{g}"] = np.ascontiguousarray(
                    w1packs[e][:, :, lo:hi]
                ).reshape(128, -1)
            for mm in range(KD):
                m[f"W2{key}{mm}"] = np.ascontiguousarray(
                    w2packs[e][:, :, mm * 128 : (mm + 1) * 128]
                ).reshape(128, -1)
            m[f"b1{key}"] = np.ascontiguousarray(
                np.asarray(b1[e], np.float32).reshape(KF, 128).T
            )
            m[f"b2{key}"] = np.ascontiguousarray(
                np.asarray(b2[e], np.float32).reshape(KD, 128).T
            )
        in_maps.append(m)
    return x, slot_tokens, q, r, in_maps


def _run(x, Wg, bg, W1, b1, W2, b2, **run_kwargs):
    x, slot_tokens, q, r, in_maps = _prepare(x, Wg, bg, W1, b1, W2, b2)
    cap = q + r
    key = (q, r)
    prog = _programs.get(key)
    if prog is None:
        prog = _programs.setdefault(key, _build_program(q, r))
    res = run_bass_kernel_spmd(
        prog, in_maps, core_ids=list(range(N_EXP)), **run_kwargs
    )
    out = np.zeros_like(x)
    for c in range(N_EXP):
        yp = np.asarray(res.results[c]["yT"], np.float32)  # [128, KD*cap]
        yT = yp.reshape(128, KD, cap).transpose(1, 0, 2).reshape(D_MODEL, cap)
        for key2, off, width in (("A", 0, q), ("B", q, r)):
            toks = slot_tokens[(c, key2)]
            out[toks] += yT[:, off : off + len(toks)].T
    return out, res


def kernel(x, Wg, bg, W1, b1, W2, b2):
    out, _ = _run(x, Wg, bg, W1, b1, W2, b2)
    return out


# revision 7
# speedup vs baseline: 1.1822x; 1.0147x over previous
"""Expert-parallel MoE FFN for Trainium2 — two expert slots per NeuronCore.

Strategy
--------
The reference sums top-2 expert FFN outputs per token (binary combine mask).
Total work is TOKENS*TOP_K = 8192 token-expert pairs; a perfectly balanced
core processes 1024.  One-expert-per-core padding forces every core to the
max expert count (~1086 -> cap 1092).  Instead each core runs TWO slots
[A: q tokens | B: r tokens] with independent expert weight sets; a tiny
host-side solver picks (q, r) and an expert->slot assignment so that hot
experts span two A-slots and cold experts two B-slots.  For the graded
distribution this gives q=544, r=500, cap=1044 — 4.4% less PE time.

Device side per slot: hT = relu(W1^T-chained matmuls + b1); yT = W2-chained
matmuls + b2, contraction dims on SBUF partitions for both layers, bf16
inputs, fp32 PSUM.  Token columns are chunked <=512 to fit one PSUM bank.

DMA: both experts' weights (32 MB) stream through rotating SBUF pools in
exact consumption order on the single SWDGE (gpsimd) ring: W1A by f-groups,
W2A by m-groups (m-major so mm2's m-outer loop consumes groups in arrival
order), then W1B, W2B.  The first x/W1 groups ride the sync-queue HWDGE,
which starts generating ~3us earlier than the SWDGE ring, pulling the first
matmul from ~13.5us to ~9us.  Outputs ride the sync queue throughout.
"""

import numpy as np
import ml_dtypes

import concourse.bacc as bacc
import concourse.mybir as mybir
import concourse.tile as tile
from concourse.bass_utils import run_bass_kernel_spmd
from concourse._compat import get_trn_type

D_MODEL = 1024
D_FF = 4096
N_EXP = 8
TOP_K = 2
KD = D_MODEL // 128  # 8 contraction chunks over d_model
KF = D_FF // 128  # 32 contraction chunks over d_ff

# W1 f-column group breakpoints: small first groups so mm1 starts early.
W1_BREAKS_A = [0, 128, 384, 896, 1408, 1920, 2432, 2944, 3456, 3968, 4096]
W1_BREAKS_B = [0, 512, 1024, 1536, 2048, 2560, 3072, 3584, 4096]

BF16 = mybir.dt.bfloat16
F32 = mybir.dt.float32

_programs: dict[tuple, object] = {}


def _chunks(width):
    """Split a slot width into matmul chunks <=512 (one fp32 PSUM bank),
    each a multiple of 4 and >=128 (keeps LDWEIGHTS hidden)."""
    n = -(-width // 512)
    base = width // n
    sizes = []
    rem = width
    for i in range(n - 1):
        w = -(-base // 4) * 4
        sizes.append(w)
        rem -= w
    sizes.append(rem)
    assert sum(sizes) == width and all(
        128 <= w <= 512 and w % 4 == 0 for w in sizes
    ), (width, sizes)
    return sizes


def _build_program(q, r):
    """Bass/Tile program: two expert slots (q and r token columns)."""
    cap = q + r
    nc = bacc.Bacc(get_trn_type() or "TRN2", target_bir_lowering=False, debug=False)

    slots = [
        dict(key="A", width=q, breaks=W1_BREAKS_A),
        dict(key="B", width=r, breaks=W1_BREAKS_B),
    ]
    for s in slots:
        s["chunks"] = _chunks(s["width"])

    # ---- DRAM tensors ----------------------------------------------------
    ka = KD // 2
    xa0_d = nc.dram_tensor("xa0", [128, ka * slots[0]["chunks"][0]], BF16,
                           kind="ExternalInput").ap()
    xa1_d = nc.dram_tensor("xa1", [128, (KD - ka) * slots[0]["chunks"][0]], BF16,
                           kind="ExternalInput").ap()
    xa2_w = slots[0]["width"] - slots[0]["chunks"][0]
    xa2_d = nc.dram_tensor("xa2", [128, KD * xa2_w], BF16,
                           kind="ExternalInput").ap() if xa2_w else None
    xb_d = nc.dram_tensor("xb", [128, KD * slots[1]["width"]], BF16,
                          kind="ExternalInput").ap()
    for s in slots:
        k = s["key"]
        s["w1_d"] = [
            nc.dram_tensor(f"W1{k}{g}", [128, KD * (hi - lo)], BF16,
                           kind="ExternalInput").ap()
            for g, (lo, hi) in enumerate(zip(s["breaks"], s["breaks"][1:]))
        ]
        s["w2_d"] = [
            nc.dram_tensor(f"W2{k}{m}", [128, KF * 128], BF16,
                           kind="ExternalInput").ap()
            for m in range(KD)
        ]
        s["b1_d"] = nc.dram_tensor(f"b1{k}", [128, KF], F32, kind="ExternalInput").ap()
        s["b2_d"] = nc.dram_tensor(f"b2{k}", [128, KD], F32, kind="ExternalInput").ap()
    y_d = nc.dram_tensor("yT", [128, KD * cap], F32, kind="ExternalOutput").ap()
    y_v = y_d.rearrange("p (m c) -> p m c", c=cap)

    with tile.TileContext(nc) as tc:
        with (
            tc.tile_pool(name="sb", bufs=1) as sb,
            tc.tile_pool(name="wp", bufs=1) as wp,
            tc.tile_pool(name="hp", bufs=1) as hp,
            tc.tile_pool(name="yp", bufs=3) as yp,
            tc.tile_pool(name="pp1", bufs=5, space="PSUM") as pp1,
            tc.tile_pool(name="pp2", bufs=2, space="PSUM") as pp2,
        ):
            # ---- persistent input tiles ---------------------------------
            xa0 = sb.tile([128, xa0_d.shape[1]], BF16, tag="xa0", name="xa0_sb")
            xa1 = sb.tile([128, xa1_d.shape[1]], BF16, tag="xa1", name="xa1_sb")
            xa2 = (sb.tile([128, xa2_d.shape[1]], BF16, tag="xa2", name="xa2_sb")
                   if xa2_d is not None else None)
            xb = sb.tile([128, xb_d.shape[1]], BF16, tag="xb", name="xb_sb")
            for s in slots:
                k = s["key"]
                s["b1"] = sb.tile([128, KF], F32, tag=f"b1{k}", name=f"b1{k}_sb")
                s["b2"] = sb.tile([128, KD], F32, tag=f"b2{k}", name=f"b2{k}_sb")

            # Streaming weight tiles: rotating slots in the wp pool.
            w1_slot_cols = KD * max(
                hi - lo for s in slots
                for lo, hi in zip(s["breaks"], s["breaks"][1:])
            )
            for s in slots:
                s["w1_t"] = [
                    wp.tile([128, KD * (hi - lo)], BF16, tag="w1", bufs=10,
                            padded_shape=[128, w1_slot_cols],
                            name=f"w1{s['key']}{g}")
                    for g, (lo, hi) in enumerate(zip(s["breaks"], s["breaks"][1:]))
                ]
                s["w2_t"] = [
                    wp.tile([128, KF * 128], BF16, tag="w2", bufs=4,
                            name=f"w2{s['key']}{m}")
                    for m in range(KD)
                ]

            # ---- DMA: single SWDGE ring, exact need-order ---------------
            nc.gpsimd.dma_start(xa0[:], xa0_d)
            nc.gpsimd.dma_start(slots[0]["w1_t"][0][:], slots[0]["w1_d"][0])
            nc.gpsimd.dma_start(xa1[:], xa1_d)
            nc.gpsimd.dma_start(slots[0]["w1_t"][1][:], slots[0]["w1_d"][1])
            if xa2 is not None:
                nc.gpsimd.dma_start(xa2[:], xa2_d)
            nc.gpsimd.dma_start(slots[0]["b1"][:], slots[0]["b1_d"])
            nc.gpsimd.dma_start(slots[0]["b2"][:], slots[0]["b2_d"])
            for g in range(2, len(slots[0]["w1_t"])):
                nc.gpsimd.dma_start(slots[0]["w1_t"][g][:], slots[0]["w1_d"][g])
            for m in range(KD):
                nc.gpsimd.dma_start(slots[0]["w2_t"][m][:], slots[0]["w2_d"][m])
            nc.gpsimd.dma_start(slots[1]["b1"][:], slots[1]["b1_d"])
            nc.gpsimd.dma_start(slots[1]["b2"][:], slots[1]["b2_d"])
            nc.gpsimd.dma_start(xb[:], xb_d)
            for g in range(len(slots[1]["w1_t"])):
                nc.gpsimd.dma_start(slots[1]["w1_t"][g][:], slots[1]["w1_d"][g])
            for m in range(KD):
                nc.gpsimd.dma_start(slots[1]["w2_t"][m][:], slots[1]["w2_d"][m])

            # ---- accessors ----------------------------------------------
            def x_rhs(s, k, off, w):
                if s["key"] == "B":
                    return xb[:, k * s["width"] + off : k * s["width"] + off + w]
                c0 = s["chunks"][0]
                if off < c0:
                    t, kk = (xa0, k) if k < ka else (xa1, k - ka)
                    return t[:, kk * c0 + off : kk * c0 + off + w]
                return xa2[:, k * xa2_w + (off - c0) : k * xa2_w + (off - c0) + w]

            def w1_lhsT(s, k, f):
                col = f * 128
                for (lo, hi), t in zip(zip(s["breaks"], s["breaks"][1:]), s["w1_t"]):
                    if lo <= col < hi:
                        base = k * (hi - lo) + (col - lo)
                        return t[:, base : base + 128]
                raise AssertionError

            # ---- compute: per slot, mm1 then mm2 ------------------------
            cap_off = 0
            for s in slots:
                offs = []
                o = 0
                for w in s["chunks"]:
                    offs.append(o)
                    o += w
                # mm1 runs chunk-by-chunk (not interleaved) so the very first
                # matmuls depend only on the sync-queue groups (xa0/W1g0/xa1);
                # chunk 1's x block has a full chunk-0 f-sweep of slack.
                h_tiles = [[None] * KF for _ in s["chunks"]]
                for ci, (o, w) in enumerate(zip(offs, s["chunks"])):
                    for f in range(KF):
                        ps = pp1.tile([128, w], F32, tag="ps1", bufs=5,
                                      name=f"ps1_{s['key']}_{ci}_{f}")
                        for k in range(KD):
                            nc.tensor.matmul(
                                ps[:],
                                w1_lhsT(s, k, f),
                                x_rhs(s, k, o, w),
                                start=(k == 0),
                                stop=(k == KD - 1),
                            )
                        ht = hp.tile([128, w], BF16, tag=f"h{s['key']}{ci}",
                                     bufs=KF, name=f"h_{s['key']}_{ci}_{f}")
                        nc.scalar.activation(
                            ht[:],
                            ps[:],
                            mybir.ActivationFunctionType.Relu,
                            bias=s["b1"][:, f : f + 1],
                        )
                        h_tiles[ci][f] = ht

                for m in range(KD):
                    ps2s = [
                        pp2.tile([128, w], F32, tag="ps2", bufs=2,
                                 name=f"ps2_{s['key']}_{m}_{ci}")
                        for ci, w in enumerate(s["chunks"])
                    ]
                    for f in range(KF):
                        w2t = s["w2_t"][m]
                        for ci, w in enumerate(s["chunks"]):
                            nc.tensor.matmul(
                                ps2s[ci][:],
                                w2t[:, f * 128 : (f + 1) * 128],
                                h_tiles[ci][f][:],
                                start=(f == 0),
                                stop=(f == KF - 1),
                            )
                    yt = yp.tile([128, s["width"]], F32, tag="y",
                                 padded_shape=[128, max(q, r)],
                                 name=f"y_{s['key']}_{m}")
                    for ci, (o, w) in enumerate(zip(offs, s["chunks"])):
                        nc.vector.tensor_scalar_add(
                            yt[:, o : o + w], ps2s[ci][:], s["b2"][:, m : m + 1]
                        )
                    nc.sync.dma_start(
                        y_v[:, m, cap_off : cap_off + s["width"]], yt[:, : s["width"]]
                    )
                cap_off += s["width"]

    nc.compile()
    return nc


def _gating_topk(x, Wg, bg):
    """Replicates jax.nn.softmax + jax.lax.top_k(..., 2) in fp32 numpy."""
    logits = x @ Wg + bg
    m = logits.max(axis=1, keepdims=True)
    e = np.exp(logits - m)
    scores = e / e.sum(axis=1, keepdims=True)
    order = np.argsort(-scores, axis=1, kind="stable")
    return order[:, :TOP_K]


def _solve_slots(counts):
    """Pick slot widths (q, r) and an expert assignment minimizing cap=q+r.

    Each core runs [q cols of expert a_c | r cols of expert b_c].  Expert e
    may occupy several A- and/or B-slots (n_A, n_B) as long as
    n_A*q + n_B*r >= counts[e], sum(n_A) <= 8, sum(n_B) <= 8.
    Returns (q, r, a_slots, b_slots): lists of 8 expert ids.
    """
    E = len(counts)
    cmax = max(counts)

    def feasible(qq, rr):
        # DP over experts on (sumA, sumB); keep pareto frontier with choices.
        frontier = {(0, 0): []}
        for e in range(E):
            new = {}
            for na in range(0, E + 1):
                rem = counts[e] - na * qq
                nb = 0 if rem <= 0 else (-(-rem // rr) if rr > 0 else None)
                if nb is None or nb > E:
                    continue
                for (sa, sb), ch in frontier.items():
                    key = (sa + na, sb + nb)
                    if key[0] <= E and key[1] <= E and key not in new:
                        new[key] = ch + [(na, nb)]
            # prune dominated states
            pruned = {}
            for k in sorted(new):
                if not any(k2[0] <= k[0] and k2[1] <= k[1] and k2 != k
                           for k2 in new):
                    pruned[k] = new[k]
            frontier = pruned or new
            if not frontier:
                return None
        best = min(((sa, sb) for sa, sb in frontier if sa <= 8 and sb <= 8),
                   default=None)
        return frontier[best] if best else None

    up4 = lambda v: -(-v // 4) * 4
    fallback = up4(-(-cmax // 2))
    for cap in range(up4(-(-sum(counts) // E)), 2 * fallback + 8, 4):
        for qq in range(up4(-(-cap // 2)), cap - 127, 4):
            rr = cap - qq
            if rr < 128 or qq > 1024 or rr > 1024:
                continue
            choice = feasible(qq, rr)
            if choice:
                a_slots, b_slots = [], []
                for e, (na, nb) in enumerate(choice):
                    a_slots += [e] * na
                    b_slots += [e] * nb
                a_slots += [a_slots[-1]] * (8 - len(a_slots))
                b_slots += [b_slots[-1]] * (8 - len(b_slots))
                return qq, rr, a_slots, b_slots
    # guaranteed fallback: every core single-expert across both slots
    q = r = fallback
    return q, r, list(range(E)), list(range(E))


def _pack_k128(a):
    """[K*128, F] -> [128, K*F]: partition-major packing of the SBUF layout."""
    k128, f = a.shape
    return np.ascontiguousarray(
        a.reshape(k128 // 128, 128, f).transpose(1, 0, 2).reshape(128, -1)
    )


def _prepare(x, Wg, bg, W1, b1, W2, b2):
    x = np.ascontiguousarray(np.asarray(x, dtype=np.float32))
    topk = _gating_topk(x, np.asarray(Wg, np.float32), np.asarray(bg, np.float32))
    idx = [np.nonzero((topk == e).any(axis=1))[0] for e in range(N_EXP)]
    counts = [len(i) for i in idx]
    q, r, a_slots, b_slots = _solve_slots(counts)
    # Process the (typically wider-chunk) r-slot first: device slot A gets
    # the single wide chunk, keeping the PE's early LDWEIGHTS duty low.
    q, r, a_slots, b_slots = r, q, b_slots, a_slots

    # Distribute each expert's tokens over its slots (A-slots first).
    slot_tokens = {}  # (core, 'A'|'B') -> token ids
    cursor = [0] * N_EXP
    for key, width, owners in (("A", q, a_slots), ("B", r, b_slots)):
        for c, e in enumerate(owners):
            t0 = cursor[e]
            t1 = min(t0 + width, counts[e])
            slot_tokens[(c, key)] = idx[e][t0:t1]
            cursor[e] = t1
    assert all(cursor[e] >= counts[e] for e in range(N_EXP)), (
        counts, q, r, a_slots, b_slots)

    bf16 = ml_dtypes.bfloat16
    w1packs, w2packs = {}, {}
    for e in set(a_slots) | set(b_slots):
        w1 = np.asarray(W1[e], np.float32).astype(bf16)
        w1packs[e] = _pack_k128(w1).reshape(128, KD, D_FF)
        w2 = np.asarray(W2[e], np.float32).astype(bf16)
        w2packs[e] = _pack_k128(w2).reshape(128, KF, D_MODEL)

    chunks_a = _chunks(q)
    ka = KD // 2
    xa2_w = q - chunks_a[0]
    in_maps = []
    for c in range(N_EXP):
        m = {}
        for key, width, owners, breaks in (
            ("A", q, a_slots, W1_BREAKS_A),
            ("B", r, b_slots, W1_BREAKS_B),
        ):
            e = owners[c]
            toks = slot_tokens[(c, key)]
            xg = np.zeros((width, D_MODEL), np.float32)
            xg[: len(toks)] = x[toks]
            xT = np.ascontiguousarray(xg.T).astype(bf16)
            xTp = _pack_k128(xT).reshape(128, KD, width)
            if key == "A":
                c0 = chunks_a[0]
                m["xa0"] = np.ascontiguousarray(xTp[:, :ka, :c0]).reshape(128, -1)
                m["xa1"] = np.ascontiguousarray(xTp[:, ka:, :c0]).reshape(128, -1)
                if xa2_w:
                    m["xa2"] = np.ascontiguousarray(xTp[:, :, c0:]).reshape(128, -1)
            else:
                m["xb"] = np.ascontiguousarray(xTp).reshape(128, -1)
            for g, (lo, hi) in enumerate(zip(breaks, breaks[1:])):
                m[f"W1{key}{g}"] = np.ascontiguousarray(
                    w1packs[e][:, :, lo:hi]
                ).reshape(128, -1)
            for mm in range(KD):
                m[f"W2{key}{mm}"] = np.ascontiguousarray(
                    w2packs[e][:, :, mm * 128 : (mm + 1) * 128]
                ).reshape(128, -1)
            m[f"b1{key}"] = np.ascontiguousarray(
                np.asarray(b1[e], np.float32).reshape(KF, 128).T
            )
            m[f"b2{key}"] = np.ascontiguousarray(
                np.asarray(b2[e], np.float32).reshape(KD, 128).T
            )
        in_maps.append(m)
    return x, slot_tokens, q, r, in_maps


def _run(x, Wg, bg, W1, b1, W2, b2, **run_kwargs):
    x, slot_tokens, q, r, in_maps = _prepare(x, Wg, bg, W1, b1, W2, b2)
    cap = q + r
    key = (q, r)
    prog = _programs.get(key)
    if prog is None:
        prog = _programs.setdefault(key, _build_program(q, r))
    res = run_bass_kernel_spmd(
        prog, in_maps, core_ids=list(range(N_EXP)), **run_kwargs
    )
    out = np.zeros_like(x)
    for c in range(N_EXP):
        yp = np.asarray(res.results[c]["yT"], np.float32)  # [128, KD*cap]
        yT = yp.reshape(128, KD, cap).transpose(1, 0, 2).reshape(D_MODEL, cap)
        for key2, off, width in (("A", 0, q), ("B", q, r)):
            toks = slot_tokens[(c, key2)]
            out[toks] += yT[:, off : off + len(toks)].T
    return out, res


def kernel(x, Wg, bg, W1, b1, W2, b2):
    out, _ = _run(x, Wg, bg, W1, b1, W2, b2)
    return out


# revision 8
# speedup vs baseline: 1.1912x; 1.0076x over previous
"""Expert-parallel MoE FFN for Trainium2 — two expert slots per NeuronCore.

Strategy
--------
The reference sums top-2 expert FFN outputs per token (binary combine mask).
Total work is TOKENS*TOP_K = 8192 token-expert pairs; a perfectly balanced
core processes 1024.  One-expert-per-core padding forces every core to the
max expert count (~1086 -> cap 1092).  Instead each core runs TWO slots
[A: q tokens | B: r tokens] with independent expert weight sets; a tiny
host-side solver picks (q, r) and an expert->slot assignment so that hot
experts span two A-slots and cold experts two B-slots.  For the graded
distribution this gives q=544, r=500, cap=1044 — 4.4% less PE time.

Device side per slot: hT = relu(W1^T-chained matmuls + b1); yT = W2-chained
matmuls + b2, contraction dims on SBUF partitions for both layers, bf16
inputs, fp32 PSUM.  Token columns are chunked <=512 to fit one PSUM bank.

DMA: both experts' weights (32 MB) stream through rotating SBUF pools in
exact consumption order on the single SWDGE (gpsimd) ring: W1A by f-groups,
W2A by m-groups (m-major so mm2's m-outer loop consumes groups in arrival
order), then W1B, W2B.  The first x/W1 groups ride the sync-queue HWDGE,
which starts generating ~3us earlier than the SWDGE ring, pulling the first
matmul from ~13.5us to ~9us.  Outputs ride the sync queue throughout.
"""

import numpy as np
import ml_dtypes

import concourse.bacc as bacc
import concourse.mybir as mybir
import concourse.tile as tile
from concourse.bass_utils import run_bass_kernel_spmd
from concourse._compat import get_trn_type

D_MODEL = 1024
D_FF = 4096
N_EXP = 8
TOP_K = 2
KD = D_MODEL // 128  # 8 contraction chunks over d_model
KF = D_FF // 128  # 32 contraction chunks over d_ff

# W1 f-column group breakpoints: small first groups so mm1 starts early.
W1_BREAKS_A = [0, 128, 384, 896, 1408, 1920, 2432, 2944, 3456, 3968, 4096]
W1_BREAKS_B = [0, 512, 1024, 1536, 2048, 2560, 3072, 3584, 4096]

BF16 = mybir.dt.bfloat16
F32 = mybir.dt.float32

_programs: dict[tuple, object] = {}


def _chunks(width):
    """Split a slot width into matmul chunks <=512 (one fp32 PSUM bank),
    each a multiple of 4 and >=128 (keeps LDWEIGHTS hidden)."""
    n = -(-width // 512)
    base = width // n
    sizes = []
    rem = width
    for i in range(n - 1):
        w = -(-base // 4) * 4
        sizes.append(w)
        rem -= w
    sizes.append(rem)
    assert sum(sizes) == width and all(
        128 <= w <= 512 and w % 4 == 0 for w in sizes
    ), (width, sizes)
    return sizes


def _build_program(q, r):
    """Bass/Tile program: two expert slots (q and r token columns)."""
    cap = q + r
    nc = bacc.Bacc(get_trn_type() or "TRN2", target_bir_lowering=False, debug=False)

    slots = [
        dict(key="A", width=q, breaks=W1_BREAKS_A),
        dict(key="B", width=r, breaks=W1_BREAKS_B),
    ]
    for s in slots:
        s["chunks"] = _chunks(s["width"])

    # ---- DRAM tensors ----------------------------------------------------
    ka = KD // 2
    xa0_d = nc.dram_tensor("xa0", [128, ka * slots[0]["chunks"][0]], BF16,
                           kind="ExternalInput").ap()
    xa1_d = nc.dram_tensor("xa1", [128, (KD - ka) * slots[0]["chunks"][0]], BF16,
                           kind="ExternalInput").ap()
    xa2_w = slots[0]["width"] - slots[0]["chunks"][0]
    xa2_d = nc.dram_tensor("xa2", [128, KD * xa2_w], BF16,
                           kind="ExternalInput").ap() if xa2_w else None
    xb_d = nc.dram_tensor("xb", [128, KD * slots[1]["width"]], BF16,
                          kind="ExternalInput").ap()
    for s in slots:
        k = s["key"]
        s["w1_d"] = [
            nc.dram_tensor(f"W1{k}{g}", [128, KD * (hi - lo)], BF16,
                           kind="ExternalInput").ap()
            for g, (lo, hi) in enumerate(zip(s["breaks"], s["breaks"][1:]))
        ]
        s["w2_d"] = [
            nc.dram_tensor(f"W2{k}{m}", [128, KF * 128], BF16,
                           kind="ExternalInput").ap()
            for m in range(KD)
        ]
        s["b1_d"] = nc.dram_tensor(f"b1{k}", [128, KF], F32, kind="ExternalInput").ap()
        s["b2_d"] = nc.dram_tensor(f"b2{k}", [128, KD], F32, kind="ExternalInput").ap()
    y_d = nc.dram_tensor("yT", [128, KD * cap], F32, kind="ExternalOutput").ap()
    y_v = y_d.rearrange("p (m c) -> p m c", c=cap)

    with tile.TileContext(nc) as tc:
        with (
            tc.tile_pool(name="sb", bufs=1) as sb,
            tc.tile_pool(name="wp", bufs=1) as wp,
            tc.tile_pool(name="hp", bufs=1) as hp,
            tc.tile_pool(name="yp", bufs=3) as yp,
            tc.tile_pool(name="pp1", bufs=4, space="PSUM") as pp1,
            tc.tile_pool(name="pp2", bufs=4, space="PSUM") as pp2,
        ):
            # ---- persistent input tiles ---------------------------------
            xa0 = sb.tile([128, xa0_d.shape[1]], BF16, tag="xa0", name="xa0_sb")
            xa1 = sb.tile([128, xa1_d.shape[1]], BF16, tag="xa1", name="xa1_sb")
            xa2 = (sb.tile([128, xa2_d.shape[1]], BF16, tag="xa2", name="xa2_sb")
                   if xa2_d is not None else None)
            xb = sb.tile([128, xb_d.shape[1]], BF16, tag="xb", name="xb_sb")
            for s in slots:
                k = s["key"]
                s["b1"] = sb.tile([128, KF], F32, tag=f"b1{k}", name=f"b1{k}_sb")
                s["b2"] = sb.tile([128, KD], F32, tag=f"b2{k}", name=f"b2{k}_sb")

            # Streaming weight tiles: rotating slots in the wp pool.
            w1_slot_cols = KD * max(
                hi - lo for s in slots
                for lo, hi in zip(s["breaks"], s["breaks"][1:])
            )
            for s in slots:
                s["w1_t"] = [
                    wp.tile([128, KD * (hi - lo)], BF16, tag="w1", bufs=9,
                            padded_shape=[128, w1_slot_cols],
                            name=f"w1{s['key']}{g}")
                    for g, (lo, hi) in enumerate(zip(s["breaks"], s["breaks"][1:]))
                ]
                s["w2_t"] = [
                    wp.tile([128, KF * 128], BF16, tag="w2", bufs=5,
                            name=f"w2{s['key']}{m}")
                    for m in range(KD)
                ]

            # ---- DMA: first operands on the early-starting sync queue,
            # everything else on the single SWDGE ring in need-order -------
            nc.sync.dma_start(xa0[:], xa0_d)
            nc.sync.dma_start(slots[0]["w1_t"][0][:], slots[0]["w1_d"][0])
            nc.sync.dma_start(xa1[:], xa1_d)
            nc.gpsimd.dma_start(slots[0]["w1_t"][1][:], slots[0]["w1_d"][1])
            if xa2 is not None:
                nc.gpsimd.dma_start(xa2[:], xa2_d)
            nc.gpsimd.dma_start(slots[0]["b1"][:], slots[0]["b1_d"])
            nc.gpsimd.dma_start(slots[0]["b2"][:], slots[0]["b2_d"])
            for g in range(2, len(slots[0]["w1_t"])):
                nc.gpsimd.dma_start(slots[0]["w1_t"][g][:], slots[0]["w1_d"][g])
            for m in range(KD):
                nc.gpsimd.dma_start(slots[0]["w2_t"][m][:], slots[0]["w2_d"][m])
            nc.gpsimd.dma_start(slots[1]["b1"][:], slots[1]["b1_d"])
            nc.gpsimd.dma_start(slots[1]["b2"][:], slots[1]["b2_d"])
            nc.gpsimd.dma_start(xb[:], xb_d)
            for g in range(len(slots[1]["w1_t"])):
                nc.gpsimd.dma_start(slots[1]["w1_t"][g][:], slots[1]["w1_d"][g])
            for m in range(KD):
                nc.gpsimd.dma_start(slots[1]["w2_t"][m][:], slots[1]["w2_d"][m])

            # ---- accessors ----------------------------------------------
            def x_rhs(s, k, off, w):
                if s["key"] == "B":
                    return xb[:, k * s["width"] + off : k * s["width"] + off + w]
                c0 = s["chunks"][0]
                if off < c0:
                    t, kk = (xa0, k) if k < ka else (xa1, k - ka)
                    return t[:, kk * c0 + off : kk * c0 + off + w]
                return xa2[:, k * xa2_w + (off - c0) : k * xa2_w + (off - c0) + w]

            def w1_lhsT(s, k, f):
                col = f * 128
                for (lo, hi), t in zip(zip(s["breaks"], s["breaks"][1:]), s["w1_t"]):
                    if lo <= col < hi:
                        base = k * (hi - lo) + (col - lo)
                        return t[:, base : base + 128]
                raise AssertionError

            # ---- compute: per slot, mm1 then mm2 ------------------------
            cap_off = 0
            for s in slots:
                offs = []
                o = 0
                for w in s["chunks"]:
                    offs.append(o)
                    o += w
                # mm1 runs chunk-by-chunk (not interleaved) so the very first
                # matmuls depend only on the sync-queue groups (xa0/W1g0/xa1);
                # chunk 1's x block has a full chunk-0 f-sweep of slack.
                h_tiles = [[None] * KF for _ in s["chunks"]]
                for ci, (o, w) in enumerate(zip(offs, s["chunks"])):
                    for f in range(KF):
                        ps = pp1.tile([128, w], F32, tag="ps1", bufs=4,
                                      name=f"ps1_{s['key']}_{ci}_{f}")
                        for k in range(KD):
                            nc.tensor.matmul(
                                ps[:],
                                w1_lhsT(s, k, f),
                                x_rhs(s, k, o, w),
                                start=(k == 0),
                                stop=(k == KD - 1),
                            )
                        ht = hp.tile([128, w], BF16, tag=f"h{s['key']}{ci}",
                                     bufs=KF, name=f"h_{s['key']}_{ci}_{f}")
                        nc.scalar.activation(
                            ht[:],
                            ps[:],
                            mybir.ActivationFunctionType.Relu,
                            bias=s["b1"][:, f : f + 1],
                        )
                        h_tiles[ci][f] = ht

                for m in range(KD):
                    ps2s = [
                        pp2.tile([128, w], F32, tag="ps2", bufs=4,
                                 name=f"ps2_{s['key']}_{m}_{ci}")
                        for ci, w in enumerate(s["chunks"])
                    ]
                    for f in range(KF):
                        w2t = s["w2_t"][m]
                        for ci, w in enumerate(s["chunks"]):
                            nc.tensor.matmul(
                                ps2s[ci][:],
                                w2t[:, f * 128 : (f + 1) * 128],
                                h_tiles[ci][f][:],
                                start=(f == 0),
                                stop=(f == KF - 1),
                            )
                    yt = yp.tile([128, s["width"]], F32, tag="y",
                                 padded_shape=[128, max(q, r)],
                                 name=f"y_{s['key']}_{m}")
                    for ci, (o, w) in enumerate(zip(offs, s["chunks"])):
                        nc.vector.tensor_scalar_add(
                            yt[:, o : o + w], ps2s[ci][:], s["b2"][:, m : m + 1]
                        )
                    nc.sync.dma_start(
                        y_v[:, m, cap_off : cap_off + s["width"]], yt[:, : s["width"]]
                    )
                cap_off += s["width"]

    nc.compile()
    return nc


def _gating_topk(x, Wg, bg):
    """Replicates jax.nn.softmax + jax.lax.top_k(..., 2) in fp32 numpy."""
    logits = x @ Wg + bg
    m = logits.max(axis=1, keepdims=True)
    e = np.exp(logits - m)
    scores = e / e.sum(axis=1, keepdims=True)
    order = np.argsort(-scores, axis=1, kind="stable")
    return order[:, :TOP_K]


def _solve_slots(counts):
    """Pick slot widths (q, r) and an expert assignment minimizing cap=q+r.

    Each core runs [q cols of expert a_c | r cols of expert b_c].  Expert e
    may occupy several A- and/or B-slots (n_A, n_B) as long as
    n_A*q + n_B*r >= counts[e], sum(n_A) <= 8, sum(n_B) <= 8.
    Returns (q, r, a_slots, b_slots): lists of 8 expert ids.
    """
    E = len(counts)
    cmax = max(counts)

    def feasible(qq, rr):
        # DP over experts on (sumA, sumB); keep pareto frontier with choices.
        frontier = {(0, 0): []}
        for e in range(E):
            new = {}
            for na in range(0, E + 1):
                rem = counts[e] - na * qq
                nb = 0 if rem <= 0 else (-(-rem // rr) if rr > 0 else None)
                if nb is None or nb > E:
                    continue
                for (sa, sb), ch in frontier.items():
                    key = (sa + na, sb + nb)
                    if key[0] <= E and key[1] <= E and key not in new:
                        new[key] = ch + [(na, nb)]
            # prune dominated states
            pruned = {}
            for k in sorted(new):
                if not any(k2[0] <= k[0] and k2[1] <= k[1] and k2 != k
                           for k2 in new):
                    pruned[k] = new[k]
            frontier = pruned or new
            if not frontier:
                return None
        best = min(((sa, sb) for sa, sb in frontier if sa <= 8 and sb <= 8),
                   default=None)
        return frontier[best] if best else None

    up4 = lambda v: -(-v // 4) * 4
    fallback = up4(-(-cmax // 2))
    for cap in range(up4(-(-sum(counts) // E)), 2 * fallback + 8, 4):
        for qq in range(up4(-(-cap // 2)), cap - 127, 4):
            rr = cap - qq
            if rr < 128 or qq > 1024 or rr > 1024:
                continue
            choice = feasible(qq, rr)
            if choice:
                a_slots, b_slots = [], []
                for e, (na, nb) in enumerate(choice):
                    a_slots += [e] * na
                    b_slots += [e] * nb
                a_slots += [a_slots[-1]] * (8 - len(a_slots))
                b_slots += [b_slots[-1]] * (8 - len(b_slots))
                return qq, rr, a_slots, b_slots
    # guaranteed fallback: every core single-expert across both slots
    q = r = fallback
    return q, r, list(range(E)), list(range(E))


def _pack_k128(a):
    """[K*128, F] -> [128, K*F]: partition-major packing of the SBUF layout."""
    k128, f = a.shape
    return np.ascontiguousarray(
        a.reshape(k128 // 128, 128, f).transpose(1, 0, 2).reshape(128, -1)
    )


def _prepare(x, Wg, bg, W1, b1, W2, b2):
    x = np.ascontiguousarray(np.asarray(x, dtype=np.float32))
    topk = _gating_topk(x, np.asarray(Wg, np.float32), np.asarray(bg, np.float32))
    idx = [np.nonzero((topk == e).any(axis=1))[0] for e in range(N_EXP)]
    counts = [len(i) for i in idx]
    q, r, a_slots, b_slots = _solve_slots(counts)
    # Process the (typically wider-chunk) r-slot first: device slot A gets
    # the single wide chunk, keeping the PE's early LDWEIGHTS duty low.
    q, r, a_slots, b_slots = r, q, b_slots, a_slots

    # Distribute each expert's tokens over its slots (A-slots first).
    slot_tokens = {}  # (core, 'A'|'B') -> token ids
    cursor = [0] * N_EXP
    for key, width, owners in (("A", q, a_slots), ("B", r, b_slots)):
        for c, e in enumerate(owners):
            t0 = cursor[e]
            t1 = min(t0 + width, counts[e])
            slot_tokens[(c, key)] = idx[e][t0:t1]
            cursor[e] = t1
    assert all(cursor[e] >= counts[e] for e in range(N_EXP)), (
        counts, q, r, a_slots, b_slots)

    bf16 = ml_dtypes.bfloat16
    w1packs, w2packs = {}, {}
    for e in set(a_slots) | set(b_slots):
        w1 = np.asarray(W1[e], np.float32).astype(bf16)
        w1packs[e] = _pack_k128(w1).reshape(128, KD, D_FF)
        w2 = np.asarray(W2[e], np.float32).astype(bf16)
        w2packs[e] = _pack_k128(w2).reshape(128, KF, D_MODEL)

    chunks_a = _chunks(q)
    ka = KD // 2
    xa2_w = q - chunks_a[0]
    in_maps = []
    for c in range(N_EXP):
        m = {}
        for key, width, owners, breaks in (
            ("A", q, a_slots, W1_BREAKS_A),
            ("B", r, b_slots, W1_BREAKS_B),
        ):
            e = owners[c]
            toks = slot_tokens[(c, key)]
            xg = np.zeros((width, D_MODEL), np.float32)
            xg[: len(toks)] = x[toks]
            xT = np.ascontiguousarray(xg.T).astype(bf16)
            xTp = _pack_k128(xT).reshape(128, KD, width)
            if key == "A":
                c0 = chunks_a[0]
                m["xa0"] = np.ascontiguousarray(xTp[:, :ka, :c0]).reshape(128, -1)
                m["xa1"] = np.ascontiguousarray(xTp[:, ka:, :c0]).reshape(128, -1)
                if xa2_w:
                    m["xa2"] = np.ascontiguousarray(xTp[:, :, c0:]).reshape(128, -1)
            else:
                m["xb"] = np.ascontiguousarray(xTp).reshape(128, -1)
            for g, (lo, hi) in enumerate(zip(breaks, breaks[1:])):
                m[f"W1{key}{g}"] = np.ascontiguousarray(
                    w1packs[e][:, :, lo:hi]
                ).reshape(128, -1)
            for mm in range(KD):
                m[f"W2{key}{mm}"] = np.ascontiguousarray(
                    w2packs[e][:, :, mm * 128 : (mm + 1) * 128]
                ).reshape(128, -1)
            m[f"b1{key}"] = np.ascontiguousarray(
                np.asarray(b1[e], np.float32).reshape(KF, 128).T
            )
            m[f"b2{key}"] = np.ascontiguousarray(
                np.asarray(b2[e], np.float32).reshape(KD, 128).T
            )
        in_maps.append(m)
    return x, slot_tokens, q, r, in_maps


def _run(x, Wg, bg, W1, b1, W2, b2, **run_kwargs):
    x, slot_tokens, q, r, in_maps = _prepare(x, Wg, bg, W1, b1, W2, b2)
    cap = q + r
    key = (q, r)
    prog = _programs.get(key)
    if prog is None:
        prog = _programs.setdefault(key, _build_program(q, r))
    res = run_bass_kernel_spmd(
        prog, in_maps, core_ids=list(range(N_EXP)), **run_kwargs
    )
    out = np.zeros_like(x)
    for c in range(N_EXP):
        yp = np.asarray(res.results[c]["yT"], np.float32)  # [128, KD*cap]
        yT = yp.reshape(128, KD, cap).transpose(1, 0, 2).reshape(D_MODEL, cap)
        for key2, off, width in (("A", 0, q), ("B", q, r)):
            toks = slot_tokens[(c, key2)]
            out[toks] += yT[:, off : off + len(toks)].T
    return out, res


def kernel(x, Wg, bg, W1, b1, W2, b2):
    out, _ = _run(x, Wg, bg, W1, b1, W2, b2)
    return out


# revision 9
# speedup vs baseline: 1.2195x; 1.0238x over previous
"""Expert-parallel MoE FFN for Trainium2 — two expert slots per NeuronCore.

Strategy
--------
The reference sums top-2 expert FFN outputs per token (binary combine mask).
Total work is TOKENS*TOP_K = 8192 token-expert pairs; a perfectly balanced
core processes 1024.  One-expert-per-core padding forces every core to the
max expert count (~1086 -> cap 1092).  Instead each core runs TWO slots
[A: q tokens | B: r tokens] with independent expert weight sets; a tiny
host-side solver picks (q, r) and an expert->slot assignment so that hot
experts span two A-slots and cold experts two B-slots.  For the graded
distribution this gives q=544, r=500, cap=1044 — 4.4% less PE time.

Device side per slot: hT = relu(W1^T-chained matmuls + b1); yT = W2-chained
matmuls + b2, contraction dims on SBUF partitions for both layers, bf16
inputs, fp32 PSUM.  Token columns are chunked <=512 to fit one PSUM bank.

DMA: both experts' weights (32 MB) stream through rotating SBUF pools in
exact consumption order on the single SWDGE (gpsimd) ring: W1A by f-groups,
W2A by m-groups (m-major so mm2's m-outer loop consumes groups in arrival
order), then W1B, W2B.  The first x/W1 groups ride the sync-queue HWDGE,
which starts generating ~3us earlier than the SWDGE ring, pulling the first
matmul from ~13.5us to ~9us.  Outputs ride the sync queue throughout.
"""

import numpy as np
import ml_dtypes

import concourse.bacc as bacc
import concourse.mybir as mybir
import concourse.tile as tile
from concourse.bass_utils import run_bass_kernel_spmd
from concourse._compat import get_trn_type

D_MODEL = 1024
D_FF = 4096
N_EXP = 8
TOP_K = 2
KD = D_MODEL // 128  # 8 contraction chunks over d_model
KF = D_FF // 128  # 32 contraction chunks over d_ff

# W1 f-column group breakpoints: small first groups so mm1 starts early.
W1_BREAKS_A = [0, 128, 384, 896, 1408, 1920, 2432, 2944, 3456, 3968, 4096]
W1_BREAKS_B = [0, 512, 1024, 1536, 2048, 2560, 3072, 3584, 4096]

BF16 = mybir.dt.bfloat16
F32 = mybir.dt.float32

_programs: dict[tuple, object] = {}


def _chunks(width):
    """Split a slot width into matmul chunks <=512 (one fp32 PSUM bank),
    each a multiple of 4 and >=128 (keeps LDWEIGHTS hidden)."""
    n = -(-width // 512)
    base = width // n
    sizes = []
    rem = width
    for i in range(n - 1):
        w = -(-base // 4) * 4
        sizes.append(w)
        rem -= w
    sizes.append(rem)
    assert sum(sizes) == width and all(
        128 <= w <= 512 and w % 4 == 0 for w in sizes
    ), (width, sizes)
    return sizes


def _build_program(q, r):
    """Bass/Tile program: two expert slots (q and r token columns)."""
    cap = q + r
    nc = bacc.Bacc(get_trn_type() or "TRN2", target_bir_lowering=False, debug=False)

    slots = [
        dict(key="A", width=q, breaks=W1_BREAKS_A),
        dict(key="B", width=r, breaks=W1_BREAKS_B),
    ]
    for s in slots:
        s["chunks"] = _chunks(s["width"])

    # ---- DRAM tensors ----------------------------------------------------
    ka = KD // 2
    xa0_d = nc.dram_tensor("xa0", [128, ka * slots[0]["chunks"][0]], BF16,
                           kind="ExternalInput").ap()
    xa1_d = nc.dram_tensor("xa1", [128, (KD - ka) * slots[0]["chunks"][0]], BF16,
                           kind="ExternalInput").ap()
    xa2_w = slots[0]["width"] - slots[0]["chunks"][0]
    xa2_d = nc.dram_tensor("xa2", [128, KD * xa2_w], BF16,
                           kind="ExternalInput").ap() if xa2_w else None
    xb_d = nc.dram_tensor("xb", [128, KD * slots[1]["width"]], BF16,
                          kind="ExternalInput").ap()
    for s in slots:
        k = s["key"]
        s["w1_d"] = [
            nc.dram_tensor(f"W1{k}{g}", [128, KD * (hi - lo)], BF16,
                           kind="ExternalInput").ap()
            for g, (lo, hi) in enumerate(zip(s["breaks"], s["breaks"][1:]))
        ]
        s["w2_d"] = [
            nc.dram_tensor(f"W2{k}{m}", [128, KF * 128], BF16,
                           kind="ExternalInput").ap()
            for m in range(KD)
        ]
        s["b1_d"] = nc.dram_tensor(f"b1{k}", [128, KF], F32, kind="ExternalInput").ap()
        s["b2_d"] = nc.dram_tensor(f"b2{k}", [128, KD], F32, kind="ExternalInput").ap()
    y_d = nc.dram_tensor("yT", [128, KD * cap], F32, kind="ExternalOutput").ap()
    y_v = y_d.rearrange("p (m c) -> p m c", c=cap)

    with tile.TileContext(nc) as tc:
        with (
            tc.tile_pool(name="sb", bufs=1) as sb,
            tc.tile_pool(name="wp", bufs=1) as wp,
            tc.tile_pool(name="hp", bufs=1) as hp,
            tc.tile_pool(name="yp", bufs=3) as yp,
            tc.tile_pool(name="pp1", bufs=4, space="PSUM") as pp1,
            tc.tile_pool(name="pp2", bufs=4, space="PSUM") as pp2,
        ):
            # ---- persistent input tiles ---------------------------------
            xa0 = sb.tile([128, xa0_d.shape[1]], BF16, tag="xa0", name="xa0_sb")
            xa1 = sb.tile([128, xa1_d.shape[1]], BF16, tag="xa1", name="xa1_sb")
            xa2 = (sb.tile([128, xa2_d.shape[1]], BF16, tag="xa2", name="xa2_sb")
                   if xa2_d is not None else None)
            xb = sb.tile([128, xb_d.shape[1]], BF16, tag="xb", name="xb_sb")
            for s in slots:
                k = s["key"]
                s["b1"] = sb.tile([128, KF], F32, tag=f"b1{k}", name=f"b1{k}_sb")
                s["b2"] = sb.tile([128, KD], F32, tag=f"b2{k}", name=f"b2{k}_sb")

            # Streaming weight tiles: rotating slots in the wp pool.
            w1_slot_cols = KD * max(
                hi - lo for s in slots
                for lo, hi in zip(s["breaks"], s["breaks"][1:])
            )
            for s in slots:
                s["w1_t"] = [
                    wp.tile([128, KD * (hi - lo)], BF16, tag="w1", bufs=9,
                            padded_shape=[128, w1_slot_cols],
                            name=f"w1{s['key']}{g}")
                    for g, (lo, hi) in enumerate(zip(s["breaks"], s["breaks"][1:]))
                ]
                s["w2_t"] = [
                    wp.tile([128, KF * 128], BF16, tag="w2", bufs=5,
                            name=f"w2{s['key']}{m}")
                    for m in range(KD)
                ]

            # ---- DMA: single SWDGE ring, exact need-order. (Putting the
            # first loads on the sync HWDGE queue was measured to start the
            # PE 1.4us earlier but starve the SWDGE ring ~10x, stalling
            # mm1's second W1 group 4us — net loss.) ----------------------
            nc.gpsimd.dma_start(xa0[:], xa0_d)
            nc.gpsimd.dma_start(slots[0]["w1_t"][0][:], slots[0]["w1_d"][0])
            nc.gpsimd.dma_start(xa1[:], xa1_d)
            nc.gpsimd.dma_start(slots[0]["w1_t"][1][:], slots[0]["w1_d"][1])
            if xa2 is not None:
                nc.gpsimd.dma_start(xa2[:], xa2_d)
            nc.gpsimd.dma_start(slots[0]["b1"][:], slots[0]["b1_d"])
            nc.gpsimd.dma_start(slots[0]["b2"][:], slots[0]["b2_d"])
            for g in range(2, len(slots[0]["w1_t"])):
                nc.gpsimd.dma_start(slots[0]["w1_t"][g][:], slots[0]["w1_d"][g])
            for m in range(KD):
                nc.gpsimd.dma_start(slots[0]["w2_t"][m][:], slots[0]["w2_d"][m])
            nc.gpsimd.dma_start(slots[1]["b1"][:], slots[1]["b1_d"])
            nc.gpsimd.dma_start(slots[1]["b2"][:], slots[1]["b2_d"])
            nc.gpsimd.dma_start(xb[:], xb_d)
            for g in range(len(slots[1]["w1_t"])):
                nc.gpsimd.dma_start(slots[1]["w1_t"][g][:], slots[1]["w1_d"][g])
            for m in range(KD):
                nc.gpsimd.dma_start(slots[1]["w2_t"][m][:], slots[1]["w2_d"][m])

            # ---- accessors ----------------------------------------------
            def x_rhs(s, k, off, w):
                if s["key"] == "B":
                    return xb[:, k * s["width"] + off : k * s["width"] + off + w]
                c0 = s["chunks"][0]
                if off < c0:
                    t, kk = (xa0, k) if k < ka else (xa1, k - ka)
                    return t[:, kk * c0 + off : kk * c0 + off + w]
                return xa2[:, k * xa2_w + (off - c0) : k * xa2_w + (off - c0) + w]

            def w1_lhsT(s, k, f):
                col = f * 128
                for (lo, hi), t in zip(zip(s["breaks"], s["breaks"][1:]), s["w1_t"]):
                    if lo <= col < hi:
                        base = k * (hi - lo) + (col - lo)
                        return t[:, base : base + 128]
                raise AssertionError

            # ---- compute: per slot, mm1 then mm2 ------------------------
            cap_off = 0
            for s in slots:
                offs = []
                o = 0
                for w in s["chunks"]:
                    offs.append(o)
                    o += w
                # mm1 runs chunk-by-chunk (not interleaved) so the very first
                # matmuls depend only on the sync-queue groups (xa0/W1g0/xa1);
                # chunk 1's x block has a full chunk-0 f-sweep of slack.
                h_tiles = [[None] * KF for _ in s["chunks"]]
                for ci, (o, w) in enumerate(zip(offs, s["chunks"])):
                    for f in range(KF):
                        ps = pp1.tile([128, w], F32, tag="ps1", bufs=4,
                                      name=f"ps1_{s['key']}_{ci}_{f}")
                        for k in range(KD):
                            nc.tensor.matmul(
                                ps[:],
                                w1_lhsT(s, k, f),
                                x_rhs(s, k, o, w),
                                start=(k == 0),
                                stop=(k == KD - 1),
                            )
                        ht = hp.tile([128, w], BF16, tag=f"h{s['key']}{ci}",
                                     bufs=KF, name=f"h_{s['key']}_{ci}_{f}")
                        nc.scalar.activation(
                            ht[:],
                            ps[:],
                            mybir.ActivationFunctionType.Relu,
                            bias=s["b1"][:, f : f + 1],
                        )
                        h_tiles[ci][f] = ht

                for m in range(KD):
                    ps2s = [
                        pp2.tile([128, w], F32, tag="ps2", bufs=4,
                                 name=f"ps2_{s['key']}_{m}_{ci}")
                        for ci, w in enumerate(s["chunks"])
                    ]
                    for f in range(KF):
                        w2t = s["w2_t"][m]
                        for ci, w in enumerate(s["chunks"]):
                            nc.tensor.matmul(
                                ps2s[ci][:],
                                w2t[:, f * 128 : (f + 1) * 128],
                                h_tiles[ci][f][:],
                                start=(f == 0),
                                stop=(f == KF - 1),
                            )
                    yt = yp.tile([128, s["width"]], F32, tag="y",
                                 padded_shape=[128, max(q, r)],
                                 name=f"y_{s['key']}_{m}")
                    for ci, (o, w) in enumerate(zip(offs, s["chunks"])):
                        nc.vector.tensor_scalar_add(
                            yt[:, o : o + w], ps2s[ci][:], s["b2"][:, m : m + 1]
                        )
                    nc.sync.dma_start(
                        y_v[:, m, cap_off : cap_off + s["width"]], yt[:, : s["width"]]
                    )
                cap_off += s["width"]

    nc.compile()
    return nc


def _gating_topk(x, Wg, bg):
    """Replicates jax.nn.softmax + jax.lax.top_k(..., 2) in fp32 numpy."""
    logits = x @ Wg + bg
    m = logits.max(axis=1, keepdims=True)
    e = np.exp(logits - m)
    scores = e / e.sum(axis=1, keepdims=True)
    order = np.argsort(-scores, axis=1, kind="stable")
    return order[:, :TOP_K]


def _solve_slots(counts):
    """Pick slot widths (q, r) and an expert assignment minimizing cap=q+r.

    Each core runs [q cols of expert a_c | r cols of expert b_c].  Expert e
    may occupy several A- and/or B-slots (n_A, n_B) as long as
    n_A*q + n_B*r >= counts[e], sum(n_A) <= 8, sum(n_B) <= 8.
    Returns (q, r, a_slots, b_slots): lists of 8 expert ids.
    """
    E = len(counts)
    cmax = max(counts)

    def feasible(qq, rr):
        # DP over experts on (sumA, sumB); keep pareto frontier with choices.
        frontier = {(0, 0): []}
        for e in range(E):
            new = {}
            for na in range(0, E + 1):
                rem = counts[e] - na * qq
                nb = 0 if rem <= 0 else (-(-rem // rr) if rr > 0 else None)
                if nb is None or nb > E:
                    continue
                for (sa, sb), ch in frontier.items():
                    key = (sa + na, sb + nb)
                    if key[0] <= E and key[1] <= E and key not in new:
                        new[key] = ch + [(na, nb)]
            # prune dominated states
            pruned = {}
            for k in sorted(new):
                if not any(k2[0] <= k[0] and k2[1] <= k[1] and k2 != k
                           for k2 in new):
                    pruned[k] = new[k]
            frontier = pruned or new
            if not frontier:
                return None
        best = min(((sa, sb) for sa, sb in frontier if sa <= 8 and sb <= 8),
                   default=None)
        return frontier[best] if best else None

    up4 = lambda v: -(-v // 4) * 4
    fallback = up4(-(-cmax // 2))
    for cap in range(up4(-(-sum(counts) // E)), 2 * fallback + 8, 4):
        for qq in range(up4(-(-cap // 2)), cap - 127, 4):
            rr = cap - qq
            if rr < 128 or qq > 1024 or rr > 1024:
                continue
            choice = feasible(qq, rr)
            if choice:
                a_slots, b_slots = [], []
                for e, (na, nb) in enumerate(choice):
                    a_slots += [e] * na
                    b_slots += [e] * nb
                a_slots += [a_slots[-1]] * (8 - len(a_slots))
                b_slots += [b_slots[-1]] * (8 - len(b_slots))
                return qq, rr, a_slots, b_slots
    # guaranteed fallback: every core single-expert across both slots
    q = r = fallback
    return q, r, list(range(E)), list(range(E))


def _pack_k128(a):
    """[K*128, F] -> [128, K*F]: partition-major packing of the SBUF layout."""
    k128, f = a.shape
    return np.ascontiguousarray(
        a.reshape(k128 // 128, 128, f).transpose(1, 0, 2).reshape(128, -1)
    )


def _prepare(x, Wg, bg, W1, b1, W2, b2):
    x = np.ascontiguousarray(np.asarray(x, dtype=np.float32))
    topk = _gating_topk(x, np.asarray(Wg, np.float32), np.asarray(bg, np.float32))
    idx = [np.nonzero((topk == e).any(axis=1))[0] for e in range(N_EXP)]
    counts = [len(i) for i in idx]
    q, r, a_slots, b_slots = _solve_slots(counts)
    # Process the (typically wider-chunk) r-slot first: device slot A gets
    # the single wide chunk, keeping the PE's early LDWEIGHTS duty low.
    q, r, a_slots, b_slots = r, q, b_slots, a_slots

    # Distribute each expert's tokens over its slots (A-slots first).
    slot_tokens = {}  # (core, 'A'|'B') -> token ids
    cursor = [0] * N_EXP
    for key, width, owners in (("A", q, a_slots), ("B", r, b_slots)):
        for c, e in enumerate(owners):
            t0 = cursor[e]
            t1 = min(t0 + width, counts[e])
            slot_tokens[(c, key)] = idx[e][t0:t1]
            cursor[e] = t1
    assert all(cursor[e] >= counts[e] for e in range(N_EXP)), (
        counts, q, r, a_slots, b_slots)

    bf16 = ml_dtypes.bfloat16
    w1packs, w2packs = {}, {}
    for e in set(a_slots) | set(b_slots):
        w1 = np.asarray(W1[e], np.float32).astype(bf16)
        w1packs[e] = _pack_k128(w1).reshape(128, KD, D_FF)
        w2 = np.asarray(W2[e], np.float32).astype(bf16)
        w2packs[e] = _pack_k128(w2).reshape(128, KF, D_MODEL)

    chunks_a = _chunks(q)
    ka = KD // 2
    xa2_w = q - chunks_a[0]
    in_maps = []
    for c in range(N_EXP):
        m = {}
        for key, width, owners, breaks in (
            ("A", q, a_slots, W1_BREAKS_A),
            ("B", r, b_slots, W1_BREAKS_B),
        ):
            e = owners[c]
            toks = slot_tokens[(c, key)]
            xg = np.zeros((width, D_MODEL), np.float32)
            xg[: len(toks)] = x[toks]
            xT = np.ascontiguousarray(xg.T).astype(bf16)
            xTp = _pack_k128(xT).reshape(128, KD, width)
            if key == "A":
                c0 = chunks_a[0]
                m["xa0"] = np.ascontiguousarray(xTp[:, :ka, :c0]).reshape(128, -1)
                m["xa1"] = np.ascontiguousarray(xTp[:, ka:, :c0]).reshape(128, -1)
                if xa2_w:
                    m["xa2"] = np.ascontiguousarray(xTp[:, :, c0:]).reshape(128, -1)
            else:
                m["xb"] = np.ascontiguousarray(xTp).reshape(128, -1)
            for g, (lo, hi) in enumerate(zip(breaks, breaks[1:])):
                m[f"W1{key}{g}"] = np.ascontiguousarray(
                    w1packs[e][:, :, lo:hi]
                ).reshape(128, -1)
            for mm in range(KD):
                m[f"W2{key}{mm}"] = np.ascontiguousarray(
                    w2packs[e][:, :, mm * 128 : (mm + 1) * 128]
                ).reshape(128, -1)
            m[f"b1{key}"] = np.ascontiguousarray(
                np.asarray(b1[e], np.float32).reshape(KF, 128).T
            )
            m[f"b2{key}"] = np.ascontiguousarray(
                np.asarray(b2[e], np.float32).reshape(KD, 128).T
            )
        in_maps.append(m)
    return x, slot_tokens, q, r, in_maps


def _run(x, Wg, bg, W1, b1, W2, b2, **run_kwargs):
    x, slot_tokens, q, r, in_maps = _prepare(x, Wg, bg, W1, b1, W2, b2)
    cap = q + r
    key = (q, r)
    prog = _programs.get(key)
    if prog is None:
        prog = _programs.setdefault(key, _build_program(q, r))
    res = run_bass_kernel_spmd(
        prog, in_maps, core_ids=list(range(N_EXP)), **run_kwargs
    )
    out = np.zeros_like(x)
    for c in range(N_EXP):
        yp = np.asarray(res.results[c]["yT"], np.float32)  # [128, KD*cap]
        yT = yp.reshape(128, KD, cap).transpose(1, 0, 2).reshape(D_MODEL, cap)
        for key2, off, width in (("A", 0, q), ("B", q, r)):
            toks = slot_tokens[(c, key2)]
            out[toks] += yT[:, off : off + len(toks)].T
    return out, res


def kernel(x, Wg, bg, W1, b1, W2, b2):
    out, _ = _run(x, Wg, bg, W1, b1, W2, b2)
    return out
